# revision 3
# baseline (speedup 1.0000x reference)
"""BiLSTM Trainium2 kernel (nn_BiLSTM): 128 segments x 128 steps, D=1024, H=512.

Sharding: 8 cores = 2 directions x 4 segment-groups (B=32 segments/core).
All cores run an IDENTICAL forward-LSTM program; the backward direction is
realized by host-side time reversal of inputs/outputs. No collectives.

Per core:
  Phase 1 (xp): pre-activations xp = W_ih^T @ x + b as 8 token-chunk matmuls
    (fp32r, N=512) stored to a DRAM scratch, time-major.
  Phase 2 (recurrence): 128 steps; per step 64 weight-stationary bf16 matmuls
    (W_hh tiles [128,128], rhs h [128,32]) accumulate gates in PSUM, then
    sigmoid/tanh (ACT) + cell update (DVE/GpSimd). c kept fp32; h kept fp32
    for output, cast to bf16 only as next-step matmul operand.

Gate layout: m = qo*4 + hc; qo in (i,f,o,g) so sigmoid gates are contiguous;
hc = hidden chunk (H=512 -> 4 chunks of 128).
"""

import numpy as np
import ml_dtypes

import concourse.bass as bass
import concourse.mybir as mybir
import concourse.tile as tile
from concourse import bacc
from concourse.bass_utils import run_bass_kernel_spmd

P = 128
B = 32            # segments per core
L = 128           # steps per segment
D = 1024
H = 512
G = 2048          # 4*H gates per direction
KD = D // P       # 8
KH = H // P       # 4
MC = G // P       # 16 gate chunks
TOK = B * L       # 4096 tokens per core
NTC = 8           # token chunks
TCH = TOK // NTC  # 512 tokens per chunk = 16 steps
STEPS_PER_TC = TCH // B  # 16

F32 = mybir.dt.float32
F32R = mybir.dt.float32r
BF16 = mybir.dt.bfloat16

AF = mybir.ActivationFunctionType

_CACHE = {}


def build_program(steps=L):
    nc = bacc.Bacc()
    xT = nc.declare_dram_parameter("xT", [P, KD, TOK], F32R, isOutput=False)
    w_ih = nc.declare_dram_parameter("w_ih", [P, KD, MC, P], F32R, isOutput=False)
    w_hh = nc.declare_dram_parameter("w_hh", [P, KH, MC, P], BF16, isOutput=False)
    bias = nc.declare_dram_parameter("bias", [P, MC], F32, isOutput=False)
    out_h = nc.declare_dram_parameter("out_h", [L, P, KH * B], F32, isOutput=True)

    ntc = (steps + STEPS_PER_TC - 1) // STEPS_PER_TC

    with tile.TileContext(nc) as tc:
        with (
            tc.tile_pool(name="wih", bufs=1) as wih_pool,
            tc.tile_pool(name="whh", bufs=1) as whh_pool,
            tc.tile_pool(name="bias_p", bufs=1) as bias_pool,
            tc.tile_pool(name="xts", bufs=2) as xt_pool,
            tc.tile_pool(name="xps", bufs=3) as xps_pool,
            tc.tile_pool(name="psx", bufs=2, space="PSUM") as psx_pool,
            tc.tile_pool(name="dram", bufs=1, space="DRAM") as dram_pool,
            tc.tile_pool(name="xpt", bufs=3) as xpt_pool,
            tc.tile_pool(name="psr", bufs=2, space="PSUM") as psr_pool,
            tc.tile_pool(name="gt", bufs=4) as g_pool,
            tc.tile_pool(name="st", bufs=4) as s_pool,
            tc.tile_pool(name="ct", bufs=1) as c_pool,
            tc.tile_pool(name="ht", bufs=2) as h_pool,
        ):
            # ---- resident weights ----
            wih_sb = wih_pool.tile([P, KD, MC, P], F32R)
            nc.sync.dma_start(wih_sb[:], w_ih[:])
            whh_sb = whh_pool.tile([P, KH, MC, P], BF16)
            nc.sync.dma_start(whh_sb[:], w_hh[:])
            bias_sb = bias_pool.tile([P, MC], F32)
            nc.sync.dma_start(bias_sb[:], bias[:])

            # ---- phase 1: xp ----
            xp_d = [dram_pool.tile([MC, P, TCH], F32, name=f"xp_d{i}") for i in range(ntc)]
            for tcix in range(ntc):
                xt_sb = xt_pool.tile([P, KD, TCH], F32R, tag="xt")
                nc.sync.dma_start(xt_sb[:], xT[:, :, tcix * TCH:(tcix + 1) * TCH])
                for m in range(MC):
                    ps = psx_pool.tile([P, TCH], F32, tag="psx")
                    for kd in range(KD):
                        nc.tensor.matmul(
                            ps[:],
                            lhsT=wih_sb[:, kd, m, :],
                            rhs=xt_sb[:, kd, :],
                            start=(kd == 0),
                            stop=(kd == KD - 1),
                        )
                    xps = xps_pool.tile([P, TCH], F32, tag="xps")
                    nc.scalar.activation(
                        xps[:], ps[:], AF.Identity, bias=bias_sb[:, m, None]
                    )
                    nc.sync.dma_start(xp_d[tcix][m], xps[:])

            # ---- phase 2: recurrence ----
            c_sb = c_pool.tile([P, KH, B], F32)
            nc.gpsimd.memset(c_sb[:], 0.0)
            h_bf = h_pool.tile([P, KH, B], BF16, tag="hbf")
            nc.gpsimd.memset(h_bf[:], 0.0)

            # m order: gates of hidden-half 0 (hc 0,1) first, then half 1
            halves = ([0, 1], [2, 3])
            ms_order = [qo * 4 + hc for hcs in halves for hc in hcs for qo in range(4)]

            for t in range(steps):
                tcix, toff = divmod(t, STEPS_PER_TC)
                xp_t = xpt_pool.tile([P, MC, B], F32, tag="xpt")
                nc.sync.dma_start(
                    xp_t[:],
                    xp_d[tcix][:, :, toff * B:(toff + 1) * B].rearrange(
                        "m p b -> p m b"
                    ),
                )
                ps = psr_pool.tile([P, MC, B], F32, tag="psr")
                # One accumulation group per step/bank: start=True clears the
                # WHOLE bank's has_written, so only the step's first MM may
                # set it; later first-touches overwrite via has_written=0.
                for kh in range(KH):
                    for m in ms_order:
                        nc.tensor.matmul(
                            ps[:, m, :],
                            lhsT=whh_sb[:, kh, m, :],
                            rhs=h_bf[:, kh, :],
                            start=(kh == 0 and m == ms_order[0]),
                            stop=(kh == KH - 1 and m == ms_order[-1]),
                        )

                h_f = h_pool.tile([P, KH, B], F32, tag="hf")
                h_bf_new = h_pool.tile([P, KH, B], BF16, tag="hbf")
                ps_v = ps.rearrange("p (qo hc) b -> p qo hc b", qo=4)
                xp_v = xp_t.rearrange("p (qo hc) b -> p qo hc b", qo=4)
                for hcs in halves:
                    h0 = hcs[0]
                    sl = slice(h0, h0 + 2)
                    # gates for this half: [P, 4(qo), 2(hc), B]
                    g_h = g_pool.tile([P, 4, 2, B], F32, tag="gh")
                    nc.vector.tensor_add(g_h[:], ps_v[:, :, sl, :], xp_v[:, :, sl, :])
                    s_ifo = s_pool.tile([P, 3, 2, B], F32, tag="sifo")
                    t_g = s_pool.tile([P, 2, B], F32, tag="tg")
                    nc.scalar.activation(t_g[:], g_h[:, 3, :, :], AF.Tanh)
                    nc.scalar.activation(s_ifo[:], g_h[:, 0:3, :, :], AF.Sigmoid)
                    # c = f*c + i*tanh(g)
                    t1 = s_pool.tile([P, 2, B], F32, tag="t1")
                    nc.vector.tensor_mul(t1[:], s_ifo[:, 1, :, :], c_sb[:, sl, :])
                    t2 = s_pool.tile([P, 2, B], F32, tag="t2")
                    nc.vector.tensor_mul(t2[:], s_ifo[:, 0, :, :], t_g[:])
                    nc.vector.tensor_add(c_sb[:, sl, :], t1[:], t2[:])
                    tc_t = s_pool.tile([P, 2, B], F32, tag="tc")
                    nc.scalar.activation(tc_t[:], c_sb[:, sl, :], AF.Tanh)
                    nc.vector.tensor_mul(h_f[:, sl, :], s_ifo[:, 2, :, :], tc_t[:])
                    nc.vector.tensor_copy(h_bf_new[:, sl, :], h_f[:, sl, :])
                nc.sync.dma_start(out_h[t], h_f.rearrange("p k b -> p (k b)"))
                h_bf = h_bf_new
    nc.finalize()
    return nc


def _get_program():
    if "nc" not in _CACHE:
        _CACHE["nc"] = build_program()
    return _CACHE["nc"]


_TQ = [0, 1, 3, 2]  # our gate order (i,f,o,g) -> torch row-block (i,f,g,o)
_ROW_IDX = np.array(
    [[_TQ[m // 4] * H + (m % 4) * P + p for p in range(P)] for m in range(MC)]
)  # [MC, P]


def _pack_weights(W_ih, W_hh, b):
    wr = W_ih[_ROW_IDX.reshape(-1)]            # [G, D] reordered
    wr = wr.reshape(MC, P, KD, P)              # [m, gcol, kd, pd]
    w_ih_p = np.ascontiguousarray(wr.transpose(3, 2, 0, 1), np.float32)
    hr = W_hh[_ROW_IDX.reshape(-1)]
    hr = hr.reshape(MC, P, KH, P)
    w_hh_p = np.ascontiguousarray(hr.transpose(3, 2, 0, 1)).astype(ml_dtypes.bfloat16)
    bias_p = np.ascontiguousarray(b[_ROW_IDX].T, np.float32)   # [P, MC]
    return w_ih_p, w_hh_p, bias_p


def _pack_x(xg):
    """xg [B, L, D] (already time-direction-adjusted) -> [P, KD, TOK] fp32."""
    x_tm = xg.transpose(1, 0, 2).reshape(TOK, D)       # tok' = t*B + b
    xT_np = x_tm.T.reshape(KD, P, TOK).transpose(1, 0, 2)
    return np.ascontiguousarray(xT_np, np.float32)


def kernel(all_embs, boundaries, W_ih_f, W_hh_f, b_f, W_ih_b, W_hh_b, b_b):
    all_embs = np.asarray(all_embs, np.float32)
    boundaries = np.asarray(boundaries)
    n_seg = boundaries.shape[0] - 1
    x = all_embs.reshape(n_seg, L, D)

    packs = {
        0: _pack_weights(np.asarray(W_ih_f, np.float32),
                         np.asarray(W_hh_f, np.float32),
                         np.asarray(b_f, np.float32)),
        1: _pack_weights(np.asarray(W_ih_b, np.float32),
                         np.asarray(W_hh_b, np.float32),
                         np.asarray(b_b, np.float32)),
    }

    in_maps = []
    for c in range(8):
        d, grp = c // 4, c % 4
        xg = x[grp * B:(grp + 1) * B]
        if d == 1:
            xg = xg[:, ::-1, :]
        w_ih_p, w_hh_p, bias_p = packs[d]
        in_maps.append({
            "xT": _pack_x(xg),
            "w_ih": w_ih_p,
            "w_hh": w_hh_p,
            "bias": bias_p,
        })

    nc = _get_program()
    res = run_bass_kernel_spmd(nc, in_maps, list(range(8)))

    out = np.empty((n_seg, L, 2 * H), np.float32)
    for c in range(8):
        d, grp = c // 4, c % 4
        oh = res.results[c]["out_h"]                      # [L, P, KH*B]
        st = oh.reshape(L, P, KH, B).transpose(3, 0, 2, 1).reshape(B, L, H)
        if d == 1:
            st = st[:, ::-1, :]
        out[grp * B:(grp + 1) * B, :, d * H:(d + 1) * H] = st
    return out.reshape(n_seg * L, 2 * H), boundaries


# revision 4
# speedup vs baseline: 1.1849x; 1.1849x over previous
"""BiLSTM Trainium2 kernel (nn_BiLSTM): 128 segments x 128 steps, D=1024, H=512.

Sharding: 8 cores = 2 directions x 4 segment-groups (B=32 segments/core).
All cores run an IDENTICAL forward-LSTM program; the backward direction is
realized by host-side time reversal of inputs/outputs. No collectives.

Per core:
  Phase 1 (xp): pre-activations xp = W_ih^T @ x + b as 8 token-chunk matmuls
    (fp32r, N=512) stored to a DRAM scratch, time-major.
  Phase 2 (recurrence): 128 steps; per step 64 weight-stationary bf16 matmuls
    (W_hh tiles [128,128], rhs h [128,32]) accumulate gates in PSUM, then
    sigmoid/tanh (ACT) + cell update (DVE/GpSimd). c kept fp32; h kept fp32
    for output, cast to bf16 only as next-step matmul operand.

Gate layout: m = qo*4 + hc; qo in (i,f,o,g) so sigmoid gates are contiguous;
hc = hidden chunk (H=512 -> 4 chunks of 128).
"""

import numpy as np
import ml_dtypes

import concourse.bass as bass
import concourse.mybir as mybir
import concourse.tile as tile
from concourse import bacc
from concourse.bass_utils import run_bass_kernel_spmd

P = 128
B = 32            # segments per core
L = 128           # steps per segment
D = 1024
H = 512
G = 2048          # 4*H gates per direction
KD = D // P       # 8
KH = H // P       # 4
MC = G // P       # 16 gate chunks
TOK = B * L       # 4096 tokens per core
NTC = 8           # token chunks
TCH = TOK // NTC  # 512 tokens per chunk = 16 steps
STEPS_PER_TC = TCH // B  # 16

F32 = mybir.dt.float32
F32R = mybir.dt.float32r
BF16 = mybir.dt.bfloat16

AF = mybir.ActivationFunctionType

_CACHE = {}


def build_program(steps=L):
    nc = bacc.Bacc()
    xT = nc.declare_dram_parameter("xT", [P, KD, TOK], F32R, isOutput=False)
    w_ih = nc.declare_dram_parameter("w_ih", [P, KD, MC, P], F32R, isOutput=False)
    w_hh = nc.declare_dram_parameter("w_hh", [P, KH, MC, P], BF16, isOutput=False)
    bias = nc.declare_dram_parameter("bias", [P, MC], F32, isOutput=False)
    out_h = nc.declare_dram_parameter("out_h", [L, P, KH * B], F32, isOutput=True)

    ntc = (steps + STEPS_PER_TC - 1) // STEPS_PER_TC

    with tile.TileContext(nc) as tc:
        with (
            tc.tile_pool(name="wih", bufs=1) as wih_pool,
            tc.tile_pool(name="whh", bufs=1) as whh_pool,
            tc.tile_pool(name="bias_p", bufs=1) as bias_pool,
            tc.tile_pool(name="xts", bufs=2) as xt_pool,
            tc.tile_pool(name="xps", bufs=3) as xps_pool,
            tc.tile_pool(name="psx", bufs=2, space="PSUM") as psx_pool,
            tc.tile_pool(name="dram", bufs=1, space="DRAM") as dram_pool,
            tc.tile_pool(name="xpt", bufs=3) as xpt_pool,
            tc.tile_pool(name="psr", bufs=2, space="PSUM") as psr_pool,
            tc.tile_pool(name="gt", bufs=4) as g_pool,
            tc.tile_pool(name="st", bufs=4) as s_pool,
            tc.tile_pool(name="ct", bufs=1) as c_pool,
            tc.tile_pool(name="ht", bufs=2) as h_pool,
        ):
            # ---- resident weights ----
            wih_sb = wih_pool.tile([P, KD, MC, P], F32R)
            nc.sync.dma_start(wih_sb[:], w_ih[:])
            whh_sb = whh_pool.tile([P, KH, MC, P], BF16)
            nc.sync.dma_start(whh_sb[:], w_hh[:])
            bias_sb = bias_pool.tile([P, MC], F32)
            nc.sync.dma_start(bias_sb[:], bias[:])

            # xp scratch: [step-in-chunk, P, MC, B] so per-step loads are
            # one contiguous 2KB run per partition.
            xp_d = [
                dram_pool.tile([STEPS_PER_TC, P, MC, B], F32, name=f"xp_d{i}")
                for i in range(ntc)
            ]

            def emit_xp_chunk(tcix):
                xt_sb = xt_pool.tile([P, KD, TCH], F32R, tag="xt")
                nc.sync.dma_start(xt_sb[:], xT[:, :, tcix * TCH:(tcix + 1) * TCH])
                for m in range(MC):
                    ps = psx_pool.tile([P, TCH], F32, tag="psx")
                    for kd in range(KD):
                        nc.tensor.matmul(
                            ps[:],
                            lhsT=wih_sb[:, kd, m, :],
                            rhs=xt_sb[:, kd, :],
                            start=(kd == 0),
                            stop=(kd == KD - 1),
                        )
                    xps = xps_pool.tile([P, TCH], F32, tag="xps")
                    nc.scalar.activation(
                        xps[:], ps[:], AF.Identity, bias=bias_sb[:, m, None]
                    )
                    nc.sync.dma_start(
                        xp_d[tcix][:, :, m, :].rearrange("t p b -> p t b"),
                        xps.rearrange("p (t b) -> p t b", b=B),
                    )

            # m order: gates of hidden-half 0 (hc 0,1) first, then half 1
            halves = ([0, 1], [2, 3])
            ms_order = [qo * 4 + hc for hcs in halves for hc in hcs for qo in range(4)]

            state = {}

            def emit_step(t):
                h_bf = state["h_bf"]
                c_sb = state["c_sb"]
                tcix, toff = divmod(t, STEPS_PER_TC)
                xp_t = xpt_pool.tile([P, MC, B], F32, tag="xpt")
                nc.sync.dma_start(xp_t[:], xp_d[tcix][toff])
                ps = psr_pool.tile([P, MC, B], F32, tag="psr")
                # One accumulation group per step/bank: start=True clears the
                # WHOLE bank's has_written, so only the step's first MM may
                # set it; later first-touches overwrite via has_written=0.
                for kh in range(KH):
                    for m in ms_order:
                        nc.tensor.matmul(
                            ps[:, m, :],
                            lhsT=whh_sb[:, kh, m, :],
                            rhs=h_bf[:, kh, :],
                            start=(kh == 0 and m == ms_order[0]),
                            stop=(kh == KH - 1 and m == ms_order[-1]),
                        )

                h_f = h_pool.tile([P, KH, B], F32, tag="hf")
                h_bf_new = h_pool.tile([P, KH, B], BF16, tag="hbf")
                ps_v = ps.rearrange("p (qo hc) b -> p qo hc b", qo=4)
                xp_v = xp_t.rearrange("p (qo hc) b -> p qo hc b", qo=4)
                for hcs in halves:
                    h0 = hcs[0]
                    sl = slice(h0, h0 + 2)
                    # gates for this half: [P, 4(qo), 2(hc), B]
                    g_h = g_pool.tile([P, 4, 2, B], F32, tag="gh")
                    nc.vector.tensor_add(g_h[:], ps_v[:, :, sl, :], xp_v[:, :, sl, :])
                    s_ifo = s_pool.tile([P, 3, 2, B], F32, tag="sifo")
                    t_g = s_pool.tile([P, 2, B], F32, tag="tg")
                    nc.scalar.activation(s_ifo[:], g_h[:, 0:3, :, :], AF.Sigmoid)
                    nc.scalar.activation(t_g[:], g_h[:, 3, :, :], AF.Tanh)
                    # c = f*c + i*tanh(g)
                    t1 = s_pool.tile([P, 2, B], F32, tag="t1")
                    nc.vector.tensor_mul(t1[:], s_ifo[:, 1, :, :], c_sb[:, sl, :])
                    t2 = s_pool.tile([P, 2, B], F32, tag="t2")
                    nc.vector.tensor_mul(t2[:], s_ifo[:, 0, :, :], t_g[:])
                    nc.vector.tensor_add(c_sb[:, sl, :], t1[:], t2[:])
                    tc_t = s_pool.tile([P, 2, B], F32, tag="tc")
                    nc.scalar.activation(tc_t[:], c_sb[:, sl, :], AF.Tanh)
                    nc.vector.tensor_mul(h_f[:, sl, :], s_ifo[:, 2, :, :], tc_t[:])
                    nc.gpsimd.tensor_copy(h_bf_new[:, sl, :], h_f[:, sl, :])
                nc.sync.dma_start(out_h[t], h_f.rearrange("p k b -> p (k b)"))
                state["h_bf"] = h_bf_new

            # ---- emission: xp chunk 0, then steps interleaved with the
            # NEXT xp chunk so xp matmuls fill recurrence PE gaps ----
            emit_xp_chunk(0)
            c_sb = c_pool.tile([P, KH, B], F32)
            nc.gpsimd.memset(c_sb[:], 0.0)
            h_bf0 = h_pool.tile([P, KH, B], BF16, tag="hbf")
            nc.gpsimd.memset(h_bf0[:], 0.0)
            state["h_bf"] = h_bf0
            state["c_sb"] = c_sb
            for tcix in range(ntc):
                for t in range(tcix * STEPS_PER_TC,
                               min(steps, (tcix + 1) * STEPS_PER_TC)):
                    emit_step(t)
                if tcix + 1 < ntc:
                    emit_xp_chunk(tcix + 1)
    nc.finalize()
    return nc


def _get_program():
    if "nc" not in _CACHE:
        _CACHE["nc"] = build_program()
    return _CACHE["nc"]


_TQ = [0, 1, 3, 2]  # our gate order (i,f,o,g) -> torch row-block (i,f,g,o)
_ROW_IDX = np.array(
    [[_TQ[m // 4] * H + (m % 4) * P + p for p in range(P)] for m in range(MC)]
)  # [MC, P]


def _pack_weights(W_ih, W_hh, b):
    wr = W_ih[_ROW_IDX.reshape(-1)]            # [G, D] reordered
    wr = wr.reshape(MC, P, KD, P)              # [m, gcol, kd, pd]
    w_ih_p = np.ascontiguousarray(wr.transpose(3, 2, 0, 1), np.float32)
    hr = W_hh[_ROW_IDX.reshape(-1)]
    hr = hr.reshape(MC, P, KH, P)
    w_hh_p = np.ascontiguousarray(hr.transpose(3, 2, 0, 1)).astype(ml_dtypes.bfloat16)
    bias_p = np.ascontiguousarray(b[_ROW_IDX].T, np.float32)   # [P, MC]
    return w_ih_p, w_hh_p, bias_p


def _pack_x(xg):
    """xg [B, L, D] (already time-direction-adjusted) -> [P, KD, TOK] fp32."""
    x_tm = xg.transpose(1, 0, 2).reshape(TOK, D)       # tok' = t*B + b
    xT_np = x_tm.T.reshape(KD, P, TOK).transpose(1, 0, 2)
    return np.ascontiguousarray(xT_np, np.float32)


def kernel(all_embs, boundaries, W_ih_f, W_hh_f, b_f, W_ih_b, W_hh_b, b_b):
    all_embs = np.asarray(all_embs, np.float32)
    boundaries = np.asarray(boundaries)
    n_seg = boundaries.shape[0] - 1
    x = all_embs.reshape(n_seg, L, D)

    packs = {
        0: _pack_weights(np.asarray(W_ih_f, np.float32),
                         np.asarray(W_hh_f, np.float32),
                         np.asarray(b_f, np.float32)),
        1: _pack_weights(np.asarray(W_ih_b, np.float32),
                         np.asarray(W_hh_b, np.float32),
                         np.asarray(b_b, np.float32)),
    }

    in_maps = []
    for c in range(8):
        d, grp = c // 4, c % 4
        xg = x[grp * B:(grp + 1) * B]
        if d == 1:
            xg = xg[:, ::-1, :]
        w_ih_p, w_hh_p, bias_p = packs[d]
        in_maps.append({
            "xT": _pack_x(xg),
            "w_ih": w_ih_p,
            "w_hh": w_hh_p,
            "bias": bias_p,
        })

    nc = _get_program()
    res = run_bass_kernel_spmd(nc, in_maps, list(range(8)))

    out = np.empty((n_seg, L, 2 * H), np.float32)
    for c in range(8):
        d, grp = c // 4, c % 4
        oh = res.results[c]["out_h"]                      # [L, P, KH*B]
        st = oh.reshape(L, P, KH, B).transpose(3, 0, 2, 1).reshape(B, L, H)
        if d == 1:
            st = st[:, ::-1, :]
        out[grp * B:(grp + 1) * B, :, d * H:(d + 1) * H] = st
    return out.reshape(n_seg * L, 2 * H), boundaries


# revision 5
# speedup vs baseline: 1.2851x; 1.0845x over previous
"""BiLSTM Trainium2 kernel (nn_BiLSTM): 128 segments x 128 steps, D=1024, H=512.

Sharding: 8 cores = 2 directions x 4 segment-groups (B=32 segments/core).
All cores run an IDENTICAL forward-LSTM program; the backward direction is
realized by host-side time reversal of inputs/outputs. No collectives.

Per core:
  Phase 1 (xp): pre-activations xp = W_ih^T @ x + b as 8 token-chunk matmuls
    (fp32r, N=512) stored to a DRAM scratch, time-major.
  Phase 2 (recurrence): 128 steps; per step 64 weight-stationary bf16 matmuls
    (W_hh tiles [128,128], rhs h [128,32]) accumulate gates in PSUM, then
    sigmoid/tanh (ACT) + cell update (DVE/GpSimd). c kept fp32; h kept fp32
    for output, cast to bf16 only as next-step matmul operand.

Gate layout: m = qo*4 + hc; qo in (i,f,o,g) so sigmoid gates are contiguous;
hc = hidden chunk (H=512 -> 4 chunks of 128).
"""

import numpy as np
import ml_dtypes

import concourse.bass as bass
import concourse.mybir as mybir
import concourse.tile as tile
from concourse import bacc
from concourse.bass_utils import run_bass_kernel_spmd

P = 128
B = 32            # segments per core
L = 128           # steps per segment
D = 1024
H = 512
G = 2048          # 4*H gates per direction
KD = D // P       # 8
KH = H // P       # 4
MC = G // P       # 16 gate chunks
TOK = B * L       # 4096 tokens per core
NTC = 8           # token chunks
TCH = TOK // NTC  # 512 tokens per chunk = 16 steps
STEPS_PER_TC = TCH // B  # 16

F32 = mybir.dt.float32
F32R = mybir.dt.float32r
BF16 = mybir.dt.bfloat16

AF = mybir.ActivationFunctionType

_CACHE = {}


def build_program(steps=L):
    nc = bacc.Bacc()
    xT = nc.declare_dram_parameter("xT", [P, KD, TOK], F32R, isOutput=False)
    w_ih = nc.declare_dram_parameter("w_ih", [P, KD, MC, P], F32R, isOutput=False)
    w_hh = nc.declare_dram_parameter("w_hh", [P, KH, MC, P], BF16, isOutput=False)
    bias = nc.declare_dram_parameter("bias", [P, MC], F32, isOutput=False)
    out_h = nc.declare_dram_parameter("out_h", [L, P, KH * B], F32, isOutput=True)

    ntc = (steps + STEPS_PER_TC - 1) // STEPS_PER_TC

    with tile.TileContext(nc) as tc:
        with (
            tc.tile_pool(name="wih", bufs=1) as wih_pool,
            tc.tile_pool(name="whh", bufs=1) as whh_pool,
            tc.tile_pool(name="bias_p", bufs=1) as bias_pool,
            tc.tile_pool(name="xts", bufs=2) as xt_pool,
            tc.tile_pool(name="xps", bufs=3) as xps_pool,
            tc.tile_pool(name="psx", bufs=2, space="PSUM") as psx_pool,
            tc.tile_pool(name="dram", bufs=1, space="DRAM") as dram_pool,
            tc.tile_pool(name="xpt", bufs=3) as xpt_pool,
            tc.tile_pool(name="psr", bufs=2, space="PSUM") as psr_pool,
            tc.tile_pool(name="gt", bufs=4) as g_pool,
            tc.tile_pool(name="st", bufs=4) as s_pool,
            tc.tile_pool(name="ct", bufs=1) as c_pool,
            tc.tile_pool(name="ht", bufs=2) as h_pool,
        ):
            # ---- resident weights ----
            wih_sb = wih_pool.tile([P, KD, MC, P], F32R)
            nc.sync.dma_start(wih_sb[:], w_ih[:])
            whh_sb = whh_pool.tile([P, KH, MC, P], BF16)
            nc.sync.dma_start(whh_sb[:], w_hh[:])
            bias_sb = bias_pool.tile([P, MC], F32)
            nc.sync.dma_start(bias_sb[:], bias[:])

            # xp scratch: [step-in-chunk, P, MC, B] so per-step loads are
            # one contiguous 2KB run per partition.
            xp_d = [
                dram_pool.tile([STEPS_PER_TC, P, MC, B], F32, name=f"xp_d{i}")
                for i in range(ntc)
            ]

            def emit_xp_chunk(tcix):
                xt_sb = xt_pool.tile([P, KD, TCH], F32R, tag="xt")
                nc.sync.dma_start(xt_sb[:], xT[:, :, tcix * TCH:(tcix + 1) * TCH])
                for m in range(MC):
                    ps = psx_pool.tile([P, TCH], F32, tag="psx")
                    for kd in range(KD):
                        nc.tensor.matmul(
                            ps[:],
                            lhsT=wih_sb[:, kd, m, :],
                            rhs=xt_sb[:, kd, :],
                            start=(kd == 0),
                            stop=(kd == KD - 1),
                        )
                    xps = xps_pool.tile([P, TCH], F32, tag="xps")
                    nc.scalar.activation(
                        xps[:], ps[:], AF.Identity, bias=bias_sb[:, m, None]
                    )
                    nc.sync.dma_start(
                        xp_d[tcix][:, :, m, :].rearrange("t p b -> p t b"),
                        xps.rearrange("p (t b) -> p t b", b=B),
                    )

            # m order: gates of hidden-half 0 (hc 0,1) first, then half 1
            halves = ([0, 1], [2, 3])
            ms_order = [qo * 4 + hc for hcs in halves for hc in hcs for qo in range(4)]

            state = {}

            def emit_step(t):
                h_bf = state["h_bf"]
                c_sb = state["c_sb"]
                tcix, toff = divmod(t, STEPS_PER_TC)
                xp_t = xpt_pool.tile([P, MC, B], F32, tag="xpt")
                nc.sync.dma_start(xp_t[:], xp_d[tcix][toff])
                ps = psr_pool.tile([P, MC, B], F32, tag="psr")
                # One accumulation group per step/bank: start=True clears the
                # WHOLE bank's has_written, so only the step's first MM may
                # set it; later first-touches overwrite via has_written=0.
                for kh in range(KH):
                    for m in ms_order:
                        nc.tensor.matmul(
                            ps[:, m, :],
                            lhsT=whh_sb[:, kh, m, :],
                            rhs=h_bf[:, kh, :],
                            start=(kh == 0 and m == ms_order[0]),
                            stop=(kh == KH - 1 and m == ms_order[-1]),
                        )

                h_f = h_pool.tile([P, KH, B], F32, tag="hf")
                h_bf_new = h_pool.tile([P, KH, B], BF16, tag="hbf")
                ps_v = ps.rearrange("p (qo hc) b -> p qo hc b", qo=4)
                xp_v = xp_t.rearrange("p (qo hc) b -> p qo hc b", qo=4)
                for hcs in halves:
                    h0 = hcs[0]
                    sl = slice(h0, h0 + 2)
                    # gates for this half: [P, 4(qo), 2(hc), B]
                    g_h = g_pool.tile([P, 4, 2, B], F32, tag="gh")
                    nc.vector.tensor_add(g_h[:], ps_v[:, :, sl, :], xp_v[:, :, sl, :])
                    s_ifo = s_pool.tile([P, 3, 2, B], F32, tag="sifo")
                    t_g = s_pool.tile([P, 2, B], F32, tag="tg")
                    nc.scalar.activation(s_ifo[:], g_h[:, 0:3, :, :], AF.Sigmoid)
                    nc.scalar.activation(t_g[:], g_h[:, 3, :, :], AF.Tanh)
                    # c = f*c + i*tanh(g)
                    t1 = s_pool.tile([P, 2, B], F32, tag="t1")
                    nc.vector.tensor_mul(t1[:], s_ifo[:, 1, :, :], c_sb[:, sl, :])
                    t2 = s_pool.tile([P, 2, B], F32, tag="t2")
                    nc.vector.tensor_mul(t2[:], s_ifo[:, 0, :, :], t_g[:])
                    nc.vector.tensor_add(c_sb[:, sl, :], t1[:], t2[:])
                    tc_t = s_pool.tile([P, 2, B], F32, tag="tc")
                    nc.scalar.activation(tc_t[:], c_sb[:, sl, :], AF.Tanh)
                    # critical-chain copy goes straight to bf16 (next step's
                    # matmul operand); fp32 h for the output DMA is computed
                    # off-chain on GpSimd.
                    nc.vector.tensor_mul(h_bf_new[:, sl, :], s_ifo[:, 2, :, :], tc_t[:])
                    nc.gpsimd.tensor_mul(h_f[:, sl, :], s_ifo[:, 2, :, :], tc_t[:])
                nc.sync.dma_start(out_h[t], h_f.rearrange("p k b -> p (k b)"))
                state["h_bf"] = h_bf_new

            # ---- emission: xp chunk 0, then steps interleaved with the
            # NEXT xp chunk so xp matmuls fill recurrence PE gaps ----
            emit_xp_chunk(0)
            c_sb = c_pool.tile([P, KH, B], F32)
            nc.gpsimd.memset(c_sb[:], 0.0)
            h_bf0 = h_pool.tile([P, KH, B], BF16, tag="hbf")
            nc.gpsimd.memset(h_bf0[:], 0.0)
            state["h_bf"] = h_bf0
            state["c_sb"] = c_sb
            for tcix in range(ntc):
                for t in range(tcix * STEPS_PER_TC,
                               min(steps, (tcix + 1) * STEPS_PER_TC)):
                    emit_step(t)
                if tcix + 1 < ntc:
                    emit_xp_chunk(tcix + 1)
    nc.finalize()
    return nc


def _get_program():
    if "nc" not in _CACHE:
        _CACHE["nc"] = build_program()
    return _CACHE["nc"]


_TQ = [0, 1, 3, 2]  # our gate order (i,f,o,g) -> torch row-block (i,f,g,o)
_ROW_IDX = np.array(
    [[_TQ[m // 4] * H + (m % 4) * P + p for p in range(P)] for m in range(MC)]
)  # [MC, P]


def _pack_weights(W_ih, W_hh, b):
    wr = W_ih[_ROW_IDX.reshape(-1)]            # [G, D] reordered
    wr = wr.reshape(MC, P, KD, P)              # [m, gcol, kd, pd]
    w_ih_p = np.ascontiguousarray(wr.transpose(3, 2, 0, 1), np.float32)
    hr = W_hh[_ROW_IDX.reshape(-1)]
    hr = hr.reshape(MC, P, KH, P)
    w_hh_p = np.ascontiguousarray(hr.transpose(3, 2, 0, 1)).astype(ml_dtypes.bfloat16)
    bias_p = np.ascontiguousarray(b[_ROW_IDX].T, np.float32)   # [P, MC]
    return w_ih_p, w_hh_p, bias_p


def _pack_x(xg):
    """xg [B, L, D] (already time-direction-adjusted) -> [P, KD, TOK] fp32."""
    x_tm = xg.transpose(1, 0, 2).reshape(TOK, D)       # tok' = t*B + b
    xT_np = x_tm.T.reshape(KD, P, TOK).transpose(1, 0, 2)
    return np.ascontiguousarray(xT_np, np.float32)


def kernel(all_embs, boundaries, W_ih_f, W_hh_f, b_f, W_ih_b, W_hh_b, b_b):
    all_embs = np.asarray(all_embs, np.float32)
    boundaries = np.asarray(boundaries)
    n_seg = boundaries.shape[0] - 1
    x = all_embs.reshape(n_seg, L, D)

    packs = {
        0: _pack_weights(np.asarray(W_ih_f, np.float32),
                         np.asarray(W_hh_f, np.float32),
                         np.asarray(b_f, np.float32)),
        1: _pack_weights(np.asarray(W_ih_b, np.float32),
                         np.asarray(W_hh_b, np.float32),
                         np.asarray(b_b, np.float32)),
    }

    in_maps = []
    for c in range(8):
        d, grp = c // 4, c % 4
        xg = x[grp * B:(grp + 1) * B]
        if d == 1:
            xg = xg[:, ::-1, :]
        w_ih_p, w_hh_p, bias_p = packs[d]
        in_maps.append({
            "xT": _pack_x(xg),
            "w_ih": w_ih_p,
            "w_hh": w_hh_p,
            "bias": bias_p,
        })

    nc = _get_program()
    res = run_bass_kernel_spmd(nc, in_maps, list(range(8)))

    out = np.empty((n_seg, L, 2 * H), np.float32)
    for c in range(8):
        d, grp = c // 4, c % 4
        oh = res.results[c]["out_h"]                      # [L, P, KH*B]
        st = oh.reshape(L, P, KH, B).transpose(3, 0, 2, 1).reshape(B, L, H)
        if d == 1:
            st = st[:, ::-1, :]
        out[grp * B:(grp + 1) * B, :, d * H:(d + 1) * H] = st
    return out.reshape(n_seg * L, 2 * H), boundaries


# revision 8
# speedup vs baseline: 1.3872x; 1.0795x over previous
"""BiLSTM Trainium2 kernel (nn_BiLSTM): 128 segments x 128 steps, D=1024, H=512.

Sharding: 8 cores = 2 directions x 4 segment-groups (B=32 segments/core).
All cores run an IDENTICAL forward-LSTM program; the backward direction is
realized by host-side time reversal of inputs/outputs. No collectives.

Per core:
  Phase 1 (xp): pre-activations xp = W_ih^T @ x + b as 8 token-chunk matmuls
    (fp32r, N=512) stored to a DRAM scratch, time-major.
  Phase 2 (recurrence): 128 steps; per step 64 weight-stationary bf16 matmuls
    (W_hh tiles [128,128], rhs h [128,32]) accumulate gates in PSUM, then
    sigmoid/tanh (ACT) + cell update (DVE/GpSimd). c kept fp32; h kept fp32
    for output, cast to bf16 only as next-step matmul operand.

Gate layout: m = qo*4 + hc; qo in (i,f,o,g) so sigmoid gates are contiguous;
hc = hidden chunk (H=512 -> 4 chunks of 128).
"""

import numpy as np
import ml_dtypes

import concourse.bass as bass
import concourse.mybir as mybir
import concourse.tile as tile
from concourse import bacc
from concourse.bass_utils import run_bass_kernel_spmd

P = 128
B = 32            # segments per core
L = 128           # steps per segment
D = 1024
H = 512
G = 2048          # 4*H gates per direction
KD = D // P       # 8
KH = H // P       # 4
MC = G // P       # 16 gate chunks
TOK = B * L       # 4096 tokens per core
NTC = 8           # token chunks
TCH = TOK // NTC  # 512 tokens per chunk = 16 steps
STEPS_PER_TC = TCH // B  # 16

F32 = mybir.dt.float32
F32R = mybir.dt.float32r
BF16 = mybir.dt.bfloat16

AF = mybir.ActivationFunctionType

_CACHE = {}


def build_program(steps=L):
    nc = bacc.Bacc()
    xT = nc.declare_dram_parameter("xT", [P, KD, TOK], F32R, isOutput=False)
    w_ih = nc.declare_dram_parameter("w_ih", [P, KD, MC, P], F32R, isOutput=False)
    w_hh = nc.declare_dram_parameter("w_hh", [P, KH, MC, P], BF16, isOutput=False)
    bias = nc.declare_dram_parameter("bias", [P, MC], F32, isOutput=False)
    out_h = nc.declare_dram_parameter("out_h", [L, P, KH * B], F32, isOutput=True)

    ntc = (steps + STEPS_PER_TC - 1) // STEPS_PER_TC

    with tile.TileContext(nc) as tc:
        with (
            tc.tile_pool(name="wih", bufs=1) as wih_pool,
            tc.tile_pool(name="whh", bufs=1) as whh_pool,
            tc.tile_pool(name="bias_p", bufs=1) as bias_pool,
            tc.tile_pool(name="xts", bufs=2) as xt_pool,
            tc.tile_pool(name="xps", bufs=3) as xps_pool,
            tc.tile_pool(name="psx", bufs=2, space="PSUM") as psx_pool,
            tc.tile_pool(name="dram", bufs=1, space="DRAM") as dram_pool,
            tc.tile_pool(name="xpt", bufs=3) as xpt_pool,
            tc.tile_pool(name="psr", bufs=2, space="PSUM") as psr_pool,
            tc.tile_pool(name="gt", bufs=4) as g_pool,
            tc.tile_pool(name="st", bufs=4) as s_pool,
            tc.tile_pool(name="ct", bufs=1) as c_pool,
            tc.tile_pool(name="ht", bufs=2) as h_pool,
        ):
            # ---- resident weights ----
            wih_sb = wih_pool.tile([P, KD, MC, P], F32R)
            nc.sync.dma_start(wih_sb[:], w_ih[:])
            whh_sb = whh_pool.tile([P, KH, MC, P], BF16)
            nc.sync.dma_start(whh_sb[:], w_hh[:])
            bias_sb = bias_pool.tile([P, MC], F32)
            nc.sync.dma_start(bias_sb[:], bias[:])

            # xp scratch: [step-in-chunk, P, MC, B] so per-step loads are
            # one contiguous 2KB run per partition.
            xp_d = [
                dram_pool.tile([STEPS_PER_TC, P, MC, B], F32, name=f"xp_d{i}")
                for i in range(ntc)
            ]

            def emit_xp_chunk(tcix):
                xt_sb = xt_pool.tile([P, KD, TCH], F32R, tag="xt")
                nc.sync.dma_start(xt_sb[:], xT[:, :, tcix * TCH:(tcix + 1) * TCH])
                for m in range(MC):
                    ps = psx_pool.tile([P, TCH], F32, tag="psx")
                    for kd in range(KD):
                        nc.tensor.matmul(
                            ps[:],
                            lhsT=wih_sb[:, kd, m, :],
                            rhs=xt_sb[:, kd, :],
                            start=(kd == 0),
                            stop=(kd == KD - 1),
                        )
                    xps = xps_pool.tile([P, TCH], F32, tag="xps")
                    nc.scalar.activation(
                        xps[:], ps[:], AF.Identity, bias=bias_sb[:, m, None]
                    )
                    nc.sync.dma_start(
                        xp_d[tcix][:, :, m, :].rearrange("t p b -> p t b"),
                        xps.rearrange("p (t b) -> p t b", b=B),
                    )

            # gate-column order: m = half*8 + qo*2 + hcl  (qo in i,f,o,g;
            # hidden chunk hc = half*2 + hcl) -> every nonlin slice is a
            # contiguous 2D AP, and each half owns a contiguous 8-block.
            state = {}

            def emit_step(t):
                h_bf = state["h_bf"]
                c_sb = state["c_sb"]
                tcix, toff = divmod(t, STEPS_PER_TC)
                xp_t = xpt_pool.tile([P, MC, B], F32, tag="xpt")
                nc.sync.dma_start(xp_t[:], xp_d[tcix][toff])
                # per-half PSUM tiles (separate banks -> finer deps: half0's
                # gates complete mid-block). One accumulation group per tile:
                # start only on the tile's first MM (whole-bank has_written
                # clear), stop on its last.
                ps_h = [
                    psr_pool.tile([P, 8, B], F32, tag=f"psr{half}",
                                  name=f"ps_h{half}")
                    for half in range(2)
                ]
                first_seen = {0: True, 1: True}
                # section 1: kh 0,1 (needs only h chunks 0/1 = half0 of t-1),
                # section 2: kh 2,3 with half0's m-blocks first.
                mm_sched = (
                    [(kh, half) for kh in (0, 1) for half in (0, 1)]
                    + [(kh, half) for half in (0, 1) for kh in (2, 3)]
                )
                last_of = {0: (3, 0), 1: (3, 1)}
                for kh, half in mm_sched:
                    for ml in range(8):
                        m = half * 8 + ml
                        nc.tensor.matmul(
                            ps_h[half][:, ml, :],
                            lhsT=whh_sb[:, kh, m, :],
                            rhs=h_bf[:, kh, :],
                            start=(first_seen[half] and ml == 0),
                            stop=((kh, half) == last_of[half] and ml == 7),
                        )
                    if first_seen[half]:
                        first_seen[half] = False

                h_f = h_pool.tile([P, KH, B], F32, tag="hf")
                h_bf_new = h_pool.tile([P, KH, B], BF16, tag="hbf")
                for half in range(2):
                    sl = slice(half * 2, half * 2 + 2)
                    g_h = g_pool.tile([P, 8, B], F32, tag="gh")
                    nc.vector.tensor_add(
                        g_h[:], ps_h[half][:],
                        xp_t[:, half * 8:(half + 1) * 8, :],
                    )
                    s_ifo = s_pool.tile([P, 6, B], F32, tag="sifo")
                    t_g = s_pool.tile([P, 2, B], F32, tag="tg")
                    nc.scalar.activation(s_ifo[:], g_h[:, 0:6, :], AF.Sigmoid)
                    nc.scalar.activation(t_g[:], g_h[:, 6:8, :], AF.Tanh)
                    # c = f*c + i*tanh(g)   (i: cols 0:2, f: 2:4, o: 4:6)
                    t1 = s_pool.tile([P, 2, B], F32, tag="t1")
                    nc.vector.tensor_mul(t1[:], s_ifo[:, 2:4, :], c_sb[:, sl, :])
                    t2 = s_pool.tile([P, 2, B], F32, tag="t2")
                    nc.vector.tensor_mul(t2[:], s_ifo[:, 0:2, :], t_g[:])
                    nc.vector.tensor_add(c_sb[:, sl, :], t1[:], t2[:])
                    tc_t = s_pool.tile([P, 2, B], F32, tag="tc")
                    nc.scalar.activation(tc_t[:], c_sb[:, sl, :], AF.Tanh)
                    # critical-chain product goes straight to bf16 (next
                    # step's matmul operand); fp32 h for the output DMA is
                    # computed off-chain on GpSimd.
                    nc.vector.tensor_mul(h_bf_new[:, sl, :], s_ifo[:, 4:6, :], tc_t[:])
                    nc.gpsimd.tensor_mul(h_f[:, sl, :], s_ifo[:, 4:6, :], tc_t[:])
                nc.sync.dma_start(out_h[t], h_f.rearrange("p k b -> p (k b)"))
                state["h_bf"] = h_bf_new

            # ---- emission: xp chunk 0, then steps interleaved with the
            # NEXT xp chunk so xp matmuls fill recurrence PE gaps ----
            emit_xp_chunk(0)
            c_sb = c_pool.tile([P, KH, B], F32)
            nc.gpsimd.memset(c_sb[:], 0.0)
            h_bf0 = h_pool.tile([P, KH, B], BF16, tag="hbf")
            nc.gpsimd.memset(h_bf0[:], 0.0)
            state["h_bf"] = h_bf0
            state["c_sb"] = c_sb
            for tcix in range(ntc):
                for t in range(tcix * STEPS_PER_TC,
                               min(steps, (tcix + 1) * STEPS_PER_TC)):
                    emit_step(t)
                if tcix + 1 < ntc:
                    emit_xp_chunk(tcix + 1)
    nc.finalize()
    return nc


def _get_program():
    if "nc" not in _CACHE:
        _CACHE["nc"] = build_program()
    return _CACHE["nc"]


_TQ = [0, 1, 3, 2]  # our gate order (i,f,o,g) -> torch row-block (i,f,g,o)


def _row_of(m, p):
    half, r = divmod(m, 8)
    qo, hcl = divmod(r, 2)
    hc = half * 2 + hcl
    return _TQ[qo] * H + hc * P + p


_ROW_IDX = np.array([[_row_of(m, p) for p in range(P)] for m in range(MC)])  # [MC, P]


def _pack_weights(W_ih, W_hh, b):
    wr = W_ih[_ROW_IDX.reshape(-1)]            # [G, D] reordered
    wr = wr.reshape(MC, P, KD, P)              # [m, gcol, kd, pd]
    w_ih_p = np.ascontiguousarray(wr.transpose(3, 2, 0, 1), np.float32)
    hr = W_hh[_ROW_IDX.reshape(-1)]
    hr = hr.reshape(MC, P, KH, P)
    w_hh_p = np.ascontiguousarray(hr.transpose(3, 2, 0, 1)).astype(ml_dtypes.bfloat16)
    bias_p = np.ascontiguousarray(b[_ROW_IDX].T, np.float32)   # [P, MC]
    return w_ih_p, w_hh_p, bias_p


def _pack_x(xg):
    """xg [B, L, D] (already time-direction-adjusted) -> [P, KD, TOK] fp32."""
    x_tm = xg.transpose(1, 0, 2).reshape(TOK, D)       # tok' = t*B + b
    xT_np = x_tm.T.reshape(KD, P, TOK).transpose(1, 0, 2)
    return np.ascontiguousarray(xT_np, np.float32)


def kernel(all_embs, boundaries, W_ih_f, W_hh_f, b_f, W_ih_b, W_hh_b, b_b):
    all_embs = np.asarray(all_embs, np.float32)
    boundaries = np.asarray(boundaries)
    n_seg = boundaries.shape[0] - 1
    x = all_embs.reshape(n_seg, L, D)

    packs = {
        0: _pack_weights(np.asarray(W_ih_f, np.float32),
                         np.asarray(W_hh_f, np.float32),
                         np.asarray(b_f, np.float32)),
        1: _pack_weights(np.asarray(W_ih_b, np.float32),
                         np.asarray(W_hh_b, np.float32),
                         np.asarray(b_b, np.float32)),
    }

    in_maps = []
    for c in range(8):
        d, grp = c // 4, c % 4
        xg = x[grp * B:(grp + 1) * B]
        if d == 1:
            xg = xg[:, ::-1, :]
        w_ih_p, w_hh_p, bias_p = packs[d]
        in_maps.append({
            "xT": _pack_x(xg),
            "w_ih": w_ih_p,
            "w_hh": w_hh_p,
            "bias": bias_p,
        })

    nc = _get_program()
    res = run_bass_kernel_spmd(nc, in_maps, list(range(8)))

    out = np.empty((n_seg, L, 2 * H), np.float32)
    for c in range(8):
        d, grp = c // 4, c % 4
        oh = res.results[c]["out_h"]                      # [L, P, KH*B]
        st = oh.reshape(L, P, KH, B).transpose(3, 0, 2, 1).reshape(B, L, H)
        if d == 1:
            st = st[:, ::-1, :]
        out[grp * B:(grp + 1) * B, :, d * H:(d + 1) * H] = st
    return out.reshape(n_seg * L, 2 * H), boundaries
